# revision 31
# baseline (speedup 1.0000x reference)
"""Trainium2 Bass kernel for DecomposingAttnProcessor (pooled component softmax
cross-attention), sharded over 8 NeuronCores along the latent-token axis S.

Math (per batch-component bc = c*B + b):
    q = x @ Wq ; k = enc @ Wk ; v = enc @ Wv           (per-head, dh = 64)
    scores = (q k^T) * dh^-0.5                          [H, S, E]
    pooled = mean_E scores ; wp = softmax_c(pooled)
    w = softmax_E(scores) * wp
    out = (w v) @ Wo + bo + x

Sharding: each core owns a 512-row slice of S for ALL batch-components; the
component softmax couples only the c axis, which stays on-core.

Dataflow per core (all matmuls contract over the SBUF partition axis):
    xT   via PE transpose;  qT = Wq^T xT (fp32r), scaled by dh^-0.5 on evict
    kT   = Wk^T encT (bf16) with fused column 160 = ksum/E  (for pooled)
    scoresT[t, s] = kT_h^T  qT_h      -> exp on ACT evict (bf16 weights)
    pooled row / denom row per (c,h) via M=1 matmuls into one [64, S] PSUM tile
    coef = softmax_c(exp pooled) / denom, broadcast over dh by GPSIMD
    aoT[dh, s] = v_h^T w  (bf16), scaled in-place by coef
    out = aoT^T @ Wo + bo + x  (bf16 matmul, fp32 residual add)
"""

import sys
from contextlib import ExitStack

sys.path.insert(0, "/opt/trn_rl_repo")

import numpy as np

import concourse.bass as bass  # noqa: E402
from concourse import bacc, mybir  # noqa: E402
from concourse.bass_utils import run_bass_kernel_spmd  # noqa: E402
from concourse.masks import make_identity  # noqa: E402
from concourse.tile import TileContext  # noqa: E402

# Problem dims (hardcoded per spec)
BC, S, D, E, H, C = 8, 4096, 1024, 160, 16, 4
B = BC // C  # 2
DH = D // H  # 64
SCALE = DH**-0.5  # 0.125
N_CORES = 8
S_LOC = S // N_CORES  # 512 rows of S per core
S_TILE = 256  # rows processed per iteration
E0, E1 = 128, E - 128  # encoder-token chunks (128 + 32)
ND = D // 128  # 8 chunks of the hidden dim

F32 = mybir.dt.float32
F32R = mybir.dt.float32r
BF16 = mybir.dt.bfloat16


def build_body(ctx, tc, d, s_loc):
    nc = tc.nc
    P = 128
    n_sc = s_loc // S_TILE

    pools = {}

    def pool(name, bufs, space="SBUF"):
        if name not in pools:
            pools[name] = ctx.enter_context(tc.tile_pool(name=name, bufs=bufs, space=space))
        return pools[name]

    const = pool("const", 1)
    wmat = pool("wmat", 1)  # Wk|Wv bf16 pairs, later reused for Wq f32
    wop = pool("wo", 1)
    enc_in = pool("enc_in", 1)
    enct_p = pool("enct", 2)
    small = pool("small", 2)
    stage = pool("stage", 2)
    ktp = pool("kt", 1)
    vp = pool("v", 1)
    xin_p = pool("xin", 1)
    xt_p = pool("xt", 2)
    qt_p = pool("qt", 1)
    w_p = pool("w", 6)
    ao_p = pool("ao", 4)
    stats = pool("stats", 1)
    coefb_p = pool("coefb", 3)
    denst_p = pool("denst", 4)
    xr_p = pool("xr", 2)
    oh_p = pool("oh", 2)

    ksb_p = pool("ksb", 1)
    dram = pool("dram", 1, space="DRAM")

    psml = pool("psml", 3, space="PSUM")  # f32 matmul accumulators (3 banks)
    pstp = pool("pstp", 2, space="PSUM")  # bf16 transpose targets (2 banks)
    psst = pool("psst", 1, space="PSUM")  # pooled + denom collectors (2 banks)
    pbig = pool("pbig", 1, space="PSUM")  # [128,512] projections (1 bank)

    # ---- constants ----
    ident = const.tile([P, P], BF16, tag="ident")
    make_identity(nc, ident)
    ones_row = const.tile([1, P], BF16, tag="ones_row")
    nc.vector.memset(ones_row, 1.0)
    bo_bf = const.tile([1, D], BF16, tag="bo_bf")
    nc.gpsimd.dma_start(out=bo_bf, in_=d["bo"])  # f32 -> bf16 cast DMA

    # ---- load Wk/Wv as bf16 pairs ----
    wkv = []
    for i in range(ND):
        t = wmat.tile([P, 2 * D], BF16, tag=f"w{i}")
        nc.gpsimd.dma_start(out=t[:, 0:D], in_=d["Wk"][128 * i : 128 * (i + 1), :])
        nc.gpsimd.dma_start(out=t[:, D : 2 * D], in_=d["Wv"][128 * i : 128 * (i + 1), :])
        wkv.append(t)

    wo = []
    for i in range(ND):
        t = wop.tile([P, D], BF16, tag=f"wo{i}")
        nc.gpsimd.dma_start(out=t, in_=d["Wo"][128 * i : 128 * (i + 1), :])
        wo.append(t)

    # ---- DRAM scratch (pool-allocated so Tile tracks spill->reload deps) ----
    kts = [dram.tile([ND, P, E + 1], BF16, tag=f"kts{bc}", name=f"kts{bc}") for bc in range(BC)]
    v0s = [dram.tile([E0, H * (DH + 1)], BF16, tag=f"v0s{bc}", name=f"v0s{bc}") for bc in range(BC)]
    v1s = [dram.tile([E1, H * (DH + 1)], BF16, tag=f"v1s{bc}", name=f"v1s{bc}") for bc in range(BC)]

    # ---- encoder phase: per bc, compute kT (+ksum/E col) and v, spill to DRAM ----
    for bc in range(BC):
        et0 = enc_in.tile([P, D], BF16, tag="et0")
        et1 = enc_in.tile([E1, D], BF16, tag="et1")
        nc.gpsimd.dma_start(out=et0, in_=d["enc"][bc, 0:E0, :])
        nc.gpsimd.dma_start(out=et1, in_=d["enc"][bc, E0:E, :])

        enct = []
        for i in range(ND):
            ps = pstp.tile([P, E], BF16, tag="pst")
            sl = slice(128 * i, 128 * (i + 1))
            nc.tensor.transpose(ps[:, 0:E0], et0[:, sl], ident)
            nc.tensor.transpose(ps[:, E0:E], et1[:, sl], ident[0:E1, 0:E1])
            t = enct_p.tile([P, E + 1], BF16, tag=f"e{i}")
            nc.scalar.activation(t[:, 0:E], ps[:, 0:E], mybir.ActivationFunctionType.Copy)
            esum = small.tile([P, 1], F32, tag="esum")
            nc.vector.tensor_reduce(esum, t[:, 0:E], axis=mybir.AxisListType.X, op=mybir.AluOpType.add)
            nc.scalar.mul(t[:, E : E + 1], esum, 1.0 / E)
            enct.append(t)

        # kT projection: [dout-chunk, E+1], bf16 matmul (psum accumulates f32)
        for j in range(ND):
            ps = psml.tile([P, E + 1], F32, tag="ps")
            for i in range(ND):
                nc.tensor.matmul(
                    ps,
                    lhsT=wkv[i][:, 128 * j : 128 * (j + 1)],
                    rhs=enct[i][:, 0 : E + 1],
                    start=(i == 0),
                    stop=(i == ND - 1),
                )
            kst = stage.tile([P, E + 1], BF16, tag="kst")
            nc.scalar.activation(kst, ps, mybir.ActivationFunctionType.Copy)
            nc.sync.dma_start(out=kts[bc][j], in_=kst)

        # v projection: natural [t, H*(dh+1)] bf16 with a ones column per head
        # (the AV matmul then emits the softmax denominator as row 64)
        for tch, (toff, tlen) in enumerate(((0, E0), (E0, E1))):
            vst = stage.tile([tlen, H * (DH + 1)], BF16, tag=f"vst{tch}")
            vst3 = vst.rearrange("t (h w) -> t h w", w=DH + 1)
            nc.vector.memset(vst3[:, :, DH : DH + 1], 1.0)
            for half in range(2):
                ps = pbig.tile([tlen, 512], F32, tag="pbig")
                for i in range(ND):
                    nc.tensor.matmul(
                        ps,
                        lhsT=enct[i][:, toff : toff + tlen],
                        rhs=wkv[i][:, D + 512 * half : D + 512 * (half + 1)],
                        start=(i == 0),
                        stop=(i == ND - 1),
                    )
                nc.scalar.activation(
                    vst3[:, 8 * half : 8 * (half + 1), 0:DH],
                    ps.rearrange("t (h w) -> t h w", w=DH),
                    mybir.ActivationFunctionType.Copy,
                )
            nc.sync.dma_start(out=(v0s if tch == 0 else v1s)[bc], in_=vst)

    # ---- load Wq (bf16) into the Wk/Wv slots ----
    wq = []
    for i in range(ND):
        t = wmat.tile([P, D], BF16, tag=f"w{i}")
        nc.gpsimd.dma_start(out=t, in_=d["Wq"][128 * i : 128 * (i + 1), :])
        wq.append(t)

    # ---- main iterations over (b, s-chunk) ----
    kt = {}
    v0 = {}
    v1 = {}
    ksb = {}
    for b in range(B):
        # (re)load this b's kv from scratch DRAM
        for c in range(C):
            bc = c * B + b
            for j in range(ND):
                t = ktp.tile([P, E + 1], BF16, tag=f"kt{c}_{j}")
                nc.sync.dma_start(out=t, in_=kts[bc][j])
                kt[(c, j)] = t
            v0[c] = vp.tile([E0, H * (DH + 1)], BF16, tag=f"v0{c}", name=f"v0{c}")
            nc.sync.dma_start(out=v0[c], in_=v0s[bc])
            v1[c] = vp.tile([E1, H * (DH + 1)], BF16, tag=f"v1{c}", name=f"v1{c}")
            nc.sync.dma_start(out=v1[c], in_=v1s[bc])
        # block-diagonal ksum/E columns for the pooled matmul: per (c, j) a
        # [128, H] tile whose cols 2j, 2j+1 hold kt's column E (zero elsewhere)
        for c in range(C):
            for j in range(ND):
                kb = ksb_p.tile([P, H], BF16, tag=f"ksb{c}_{j}", name=f"ksb{c}_{j}")
                nc.vector.memset(kb, 0.0)
                nc.vector.tensor_copy(kb[0:64, 2 * j : 2 * j + 1], kt[(c, j)][0:64, E : E + 1])
                nc.vector.tensor_copy(kb[64:128, 2 * j + 1 : 2 * j + 2], kt[(c, j)][64:128, E : E + 1])
                ksb[(c, j)] = kb

        for sc in range(n_sc):
            r0 = sc * S_TILE

            # --- A: xT and qT per component ---
            qt = {}
            for c in range(C):
                bc = c * B + b
                xin0 = xin_p.tile([P, D], BF16, tag="xin0")
                xin1 = xin_p.tile([P, D], BF16, tag="xin1")
                nc.gpsimd.dma_start(out=xin0, in_=d["x"][bc, r0 : r0 + 128, :])
                nc.gpsimd.dma_start(out=xin1, in_=d["x"][bc, r0 + 128 : r0 + 256, :])
                xt = []
                for i in range(ND):
                    ps = pstp.tile([P, S_TILE], BF16, tag="pst")
                    sl = slice(128 * i, 128 * (i + 1))
                    nc.tensor.transpose(ps[:, 0:128], xin0[:, sl], ident)
                    nc.tensor.transpose(ps[:, 128:256], xin1[:, sl], ident)
                    t = xt_p.tile([P, S_TILE], BF16, tag=f"xt{i}")
                    nc.scalar.activation(t, ps, mybir.ActivationFunctionType.Copy)
                    xt.append(t)
                for j in range(ND):
                    ps = psml.tile([P, S_TILE], F32, tag="ps")
                    for i in range(ND):
                        nc.tensor.matmul(
                            ps,
                            lhsT=wq[i][:, 128 * j : 128 * (j + 1)],
                            rhs=xt[i],
                            start=(i == 0),
                            stop=(i == ND - 1),
                        )
                    t = qt_p.tile([P, S_TILE], BF16, tag=f"qt{c}_{j}")
                    nc.scalar.mul(t, ps, SCALE)  # fold dh^-0.5 into q
                    qt[(c, j)] = t

            # --- B: scores, exp, pooled, AV(+denom row) per (c, h) ---
            # pooled/E via M=16 matmuls; matmul base partition must be 0/32/64,
            # so components 0-2 sit in ps_pool at 32c and component 3 in pp2.
            # den_sb collects per-(c,h) softmax denominators at rows 32c+h via
            # sbuf->sbuf DMA (engines cannot write single rows off-alignment).
            ps_pool = psst.tile([P, S_TILE], F32, tag="pp")
            ps_pool2 = psst.tile([H, S_TILE], F32, tag="pp2")
            den_sb = stats.tile([P, S_TILE], F32, tag="densb")

            def pooled_slot(c):
                return (ps_pool[32 * c : 32 * c + H, :]) if c < 3 else (ps_pool2[0:H, :])

            ao = {}
            for c in range(C):
                ao[c] = ao_p.tile([P, ND * S_TILE], BF16, tag="ao", name=f"ao{c}")
                for j in range(ND):
                    nc.tensor.matmul(
                        pooled_slot(c),
                        lhsT=ksb[(c, j)],
                        rhs=qt[(c, j)],
                        start=(j == 0),
                        stop=(j == ND - 1),
                    )
                for h in range(H):
                    j, hr = h // 2, 64 * (h % 2)
                    ch = c * H + h
                    lk = kt[(c, j)]
                    rq = qt[(c, j)][hr : hr + 64, :]
                    ps_a = psml.tile([P, S_TILE], F32, tag="ps")
                    nc.tensor.matmul(ps_a, lhsT=lk[hr : hr + 64, 0:E0], rhs=rq, start=True, stop=True)
                    ps_b = psml.tile([E1, S_TILE], F32, tag="ps")
                    nc.tensor.matmul(ps_b, lhsT=lk[hr : hr + 64, E0:E], rhs=rq, start=True, stop=True)
                    wa = w_p.tile([P, S_TILE], BF16, tag="wa")
                    nc.scalar.activation(wa, ps_a, mybir.ActivationFunctionType.Exp)
                    wb = w_p.tile([E1, S_TILE], BF16, tag="wb")
                    nc.scalar.activation(wb, ps_b, mybir.ActivationFunctionType.Exp)
                    # attention-value product (unnormalized); row 64 = denom
                    ps_av = psml.tile([DH + 1, S_TILE], F32, tag="ps")
                    v_sl = slice((DH + 1) * h, (DH + 1) * (h + 1))
                    nc.tensor.matmul(ps_av, lhsT=v0[c][:, v_sl], rhs=wa, start=True, stop=False)
                    nc.tensor.matmul(ps_av, lhsT=v1[c][:, v_sl], rhs=wb, start=False, stop=True)
                    nc.vector.tensor_copy(ao[c][hr : hr + 64, S_TILE * j : S_TILE * (j + 1)], ps_av[0:DH, :])
                    den_st = denst_p.tile([1, S_TILE], F32, tag="denst")
                    nc.scalar.activation(den_st, ps_av[DH : DH + 1, :], mybir.ActivationFunctionType.Copy)
                    nc.sync.dma_start(out=den_sb[32 * c + h : 32 * c + h + 1, :], in_=den_st)

            # --- C: coef = softmax_c(exp(pooled)) / denom ---
            # TensorTensor ops need identical partition ranges on HW, so every
            # per-component stat lives in its own [16, S] tile at offset 0.
            ep = [stats.tile([H, S_TILE], F32, tag=f"ep{c}", name=f"ep{c}") for c in range(C)]
            rd = [stats.tile([H, S_TILE], F32, tag=f"rd{c}", name=f"rd{c}") for c in range(C)]
            coefc = [stats.tile([H, S_TILE], F32, tag=f"coefc{c}", name=f"coefc{c}") for c in range(C)]
            for c in range(C):
                nc.scalar.activation(ep[c], pooled_slot(c), mybir.ActivationFunctionType.Exp)
                nc.vector.reciprocal(rd[c], den_sb[32 * c : 32 * c + H, :])
            sc_sum = stats.tile([H, S_TILE], F32, tag="sc")
            nc.vector.tensor_add(sc_sum, ep[0], ep[1])
            nc.vector.tensor_add(sc_sum, sc_sum, ep[2])
            nc.vector.tensor_add(sc_sum, sc_sum, ep[3])
            rs = stats.tile([H, S_TILE], F32, tag="rs")
            nc.vector.reciprocal(rs, sc_sum)
            for c in range(C):
                nc.vector.tensor_mul(coefc[c], ep[c], rs)
                nc.vector.tensor_mul(coefc[c], coefc[c], rd[c])

            # --- D: scale aoT in place by coef (broadcast over dh via a DRAM
            # bounce: SBUF sources cannot have stride-0 partitions, DRAM can).
            # One [128, S] broadcast per (c, head-pair): rows 0:64 get head 2j,
            # rows 64:128 get head 2j+1, matching the ao tile layout. ---
            coef_d = dram.tile([C * H, S_TILE], F32, tag="coefd", name="coef_d", bufs=2)
            for c in range(C):
                nc.sync.dma_start(out=coef_d[H * c : H * (c + 1), :], in_=coefc[c])
            for c in range(C):
                for j in range(ND):
                    cb = coefb_p.tile([P, S_TILE], F32, tag="cb")
                    src2 = coef_d[c * H + 2 * j : c * H + 2 * j + 2, :]
                    src2 = bass.AP(
                        tensor=src2.tensor,
                        offset=src2.offset,
                        ap=[list(src2.ap[0]), [0, 64]] + [list(a) for a in src2.ap[1:]],
                    )
                    nc.sync.dma_start(out=cb, in_=src2)
                    sl_ao = ao[c][:, S_TILE * j : S_TILE * (j + 1)]
                    nc.vector.tensor_mul(sl_ao, sl_ao, cb)

            # --- E: output projection + bias + residual ---
            for c in range(C):
                bc = c * B + b
                for m in range(2):
                    rows = slice(r0 + 128 * m, r0 + 128 * (m + 1))
                    for half in range(2):
                        cols = slice(512 * half, 512 * (half + 1))
                        ps = pbig.tile([P, 512], F32, tag="pbig")
                        nc.tensor.matmul(ps, lhsT=ones_row, rhs=bo_bf[:, cols], start=True, stop=False)
                        for i in range(ND):
                            nc.tensor.matmul(
                                ps,
                                lhsT=ao[c][:, S_TILE * i + 128 * m : S_TILE * i + 128 * (m + 1)],
                                rhs=wo[i][:, cols],
                                start=False,
                                stop=(i == ND - 1),
                            )
                        xr = xr_p.tile([P, 512], F32, tag="xr")
                        nc.sync.dma_start(out=xr, in_=d["x"][bc, rows, cols])
                        oh = oh_p.tile([P, 512], F32, tag="oh")
                        nc.vector.tensor_add(oh, ps, xr)
                        nc.sync.dma_start(out=d["out"][bc, rows, cols], in_=oh)




def build_program(s_loc=S_LOC, n_cores=N_CORES):
    nc = bacc.Bacc(trn_type="TRN2", target_bir_lowering=False, debug=False, num_devices=n_cores)
    d = {
        "x": nc.dram_tensor("x", [BC, s_loc, D], F32, kind="ExternalInput").ap(),
        "enc": nc.dram_tensor("enc", [BC, E, D], F32, kind="ExternalInput").ap(),
        "Wq": nc.dram_tensor("Wq", [D, D], F32, kind="ExternalInput").ap(),
        "Wk": nc.dram_tensor("Wk", [D, D], F32, kind="ExternalInput").ap(),
        "Wv": nc.dram_tensor("Wv", [D, D], F32, kind="ExternalInput").ap(),
        "Wo": nc.dram_tensor("Wo", [D, D], F32, kind="ExternalInput").ap(),
        "bo": nc.dram_tensor("bo", [1, D], F32, kind="ExternalInput").ap(),
        "out": nc.dram_tensor("out", [BC, s_loc, D], F32, kind="ExternalOutput").ap(),
    }
    with TileContext(nc, trace_sim=False) as tc, ExitStack() as ctx:
        build_body(ctx, tc, d, s_loc)
    nc.compile()
    return nc


def make_in_maps(hidden_states, encoder_hidden_states, Wq, Wk, Wv, Wo, bo, s_loc=S_LOC, n_cores=N_CORES):
    common = {
        "enc": np.ascontiguousarray(encoder_hidden_states, dtype=np.float32),
        "Wq": np.ascontiguousarray(Wq, dtype=np.float32),
        "Wk": np.ascontiguousarray(Wk, dtype=np.float32),
        "Wv": np.ascontiguousarray(Wv, dtype=np.float32),
        "Wo": np.ascontiguousarray(Wo, dtype=np.float32),
        "bo": np.ascontiguousarray(bo, dtype=np.float32).reshape(1, D),
    }
    return [
        {"x": np.ascontiguousarray(hidden_states[:, i * s_loc : (i + 1) * s_loc, :], dtype=np.float32), **common}
        for i in range(n_cores)
    ]


_NC = None


def kernel(hidden_states, encoder_hidden_states, Wq, Wk, Wv, Wo, bo):
    global _NC
    if _NC is None:
        _NC = build_program()
    in_maps = make_in_maps(hidden_states, encoder_hidden_states, Wq, Wk, Wv, Wo, bo)
    res = run_bass_kernel_spmd(_NC, in_maps, list(range(N_CORES))).results
    out = np.concatenate([res[i]["out"] for i in range(N_CORES)], axis=1)
    return np.ascontiguousarray(out, dtype=np.float32)


if __name__ == "__main__":
    rng = np.random.default_rng(0)
    ins = {
        "hidden_states": rng.standard_normal((BC, S, D), dtype=np.float32),
        "encoder_hidden_states": rng.standard_normal((BC, E, D), dtype=np.float32),
        "Wq": rng.standard_normal((D, D), dtype=np.float32) * 0.02,
        "Wk": rng.standard_normal((D, D), dtype=np.float32) * 0.02,
        "Wv": rng.standard_normal((D, D), dtype=np.float32) * 0.02,
        "Wo": rng.standard_normal((D, D), dtype=np.float32) * 0.02,
        "bo": np.zeros((D,), np.float32),
    }
    out = kernel(**ins)
    print("out", out.shape, out.dtype, float(np.abs(out).max()))


# revision 32
# speedup vs baseline: 1.1083x; 1.1083x over previous
"""Trainium2 Bass kernel for DecomposingAttnProcessor (pooled component softmax
cross-attention), sharded over 8 NeuronCores along the latent-token axis S.

Math (per batch-component bc = c*B + b):
    q = x @ Wq ; k = enc @ Wk ; v = enc @ Wv           (per-head, dh = 64)
    scores = (q k^T) * dh^-0.5                          [H, S, E]
    pooled = mean_E scores ; wp = softmax_c(pooled)
    w = softmax_E(scores) * wp
    out = (w v) @ Wo + bo + x

Sharding: each core owns a 512-row slice of S for ALL batch-components; the
component softmax couples only the c axis, which stays on-core.

Dataflow per core (all matmuls contract over the SBUF partition axis):
    xT   via PE transpose;  qT = Wq^T xT (fp32r), scaled by dh^-0.5 on evict
    kT   = Wk^T encT (bf16) with fused column 160 = ksum/E  (for pooled)
    scoresT[t, s] = kT_h^T  qT_h      -> exp on ACT evict (bf16 weights)
    pooled row / denom row per (c,h) via M=1 matmuls into one [64, S] PSUM tile
    coef = softmax_c(exp pooled) / denom, broadcast over dh by GPSIMD
    aoT[dh, s] = v_h^T w  (bf16), scaled in-place by coef
    out = aoT^T @ Wo + bo + x  (bf16 matmul, fp32 residual add)
"""

import sys
from contextlib import ExitStack

sys.path.insert(0, "/opt/trn_rl_repo")

import numpy as np

import concourse.bass as bass  # noqa: E402
from concourse import bacc, mybir  # noqa: E402
from concourse.bass_utils import run_bass_kernel_spmd  # noqa: E402
from concourse.masks import make_identity  # noqa: E402
from concourse.tile import TileContext  # noqa: E402

# Problem dims (hardcoded per spec)
BC, S, D, E, H, C = 8, 4096, 1024, 160, 16, 4
B = BC // C  # 2
DH = D // H  # 64
SCALE = DH**-0.5  # 0.125
N_CORES = 8
S_LOC = S // N_CORES  # 512 rows of S per core
S_TILE = 256  # rows processed per iteration
E0, E1 = 128, E - 128  # encoder-token chunks (128 + 32)
ND = D // 128  # 8 chunks of the hidden dim

F32 = mybir.dt.float32
F32R = mybir.dt.float32r
BF16 = mybir.dt.bfloat16


def build_body(ctx, tc, d, s_loc):
    nc = tc.nc
    P = 128
    n_sc = s_loc // S_TILE

    pools = {}

    def pool(name, bufs, space="SBUF"):
        if name not in pools:
            pools[name] = ctx.enter_context(tc.tile_pool(name=name, bufs=bufs, space=space))
        return pools[name]

    const = pool("const", 1)
    wmat = pool("wmat", 1)  # Wk|Wv bf16 pairs, later reused for Wq f32
    wop = pool("wo", 1)
    enc_in = pool("enc_in", 1)
    enct_p = pool("enct", 2)
    small = pool("small", 2)
    stage = pool("stage", 2)
    ktp = pool("kt", 1)
    vp = pool("v", 1)
    xin_p = pool("xin", 2)
    xt_p = pool("xt", 2)
    qt_p = pool("qt", 1)
    w_p = pool("w", 6)
    ao_p = pool("ao", 4)
    stats = pool("stats", 1)
    coefb_p = pool("coefb", 3)
    denst_p = pool("denst", 4)
    xr_p = pool("xr", 2)
    oh_p = pool("oh", 2)

    ksb_p = pool("ksb", 1)
    dram = pool("dram", 1, space="DRAM")

    psml = pool("psml", 4, space="PSUM")  # f32 matmul accumulators (3 banks)
    pstp = pool("pstp", 1, space="PSUM")  # bf16 transpose targets (2 banks)
    psst = pool("psst", 1, space="PSUM")  # pooled + denom collectors (2 banks)
    pbig = pool("pbig", 1, space="PSUM")  # [128,512] projections (1 bank)

    # ---- constants ----
    ident = const.tile([P, P], BF16, tag="ident")
    make_identity(nc, ident)
    ones_row = const.tile([1, P], BF16, tag="ones_row")
    nc.vector.memset(ones_row, 1.0)
    bo_bf = const.tile([1, D], BF16, tag="bo_bf")
    nc.gpsimd.dma_start(out=bo_bf, in_=d["bo"])  # f32 -> bf16 cast DMA

    # ---- load Wk/Wv as bf16 pairs ----
    wkv = []
    for i in range(ND):
        t = wmat.tile([P, 2 * D], BF16, tag=f"w{i}")
        nc.gpsimd.dma_start(out=t[:, 0:D], in_=d["Wk"][128 * i : 128 * (i + 1), :])
        nc.gpsimd.dma_start(out=t[:, D : 2 * D], in_=d["Wv"][128 * i : 128 * (i + 1), :])
        wkv.append(t)

    wo = []
    for i in range(ND):
        t = wop.tile([P, D], BF16, tag=f"wo{i}")
        nc.gpsimd.dma_start(out=t, in_=d["Wo"][128 * i : 128 * (i + 1), :])
        wo.append(t)

    # ---- DRAM scratch (pool-allocated so Tile tracks spill->reload deps) ----
    kts = [dram.tile([ND, P, E + 1], BF16, tag=f"kts{bc}", name=f"kts{bc}") for bc in range(BC)]
    v0s = [dram.tile([E0, H * (DH + 1)], BF16, tag=f"v0s{bc}", name=f"v0s{bc}") for bc in range(BC)]
    v1s = [dram.tile([E1, H * (DH + 1)], BF16, tag=f"v1s{bc}", name=f"v1s{bc}") for bc in range(BC)]

    # ---- encoder phase: per bc, compute kT (+ksum/E col) and v, spill to DRAM ----
    for bc in range(BC):
        et0 = enc_in.tile([P, D], BF16, tag="et0")
        et1 = enc_in.tile([E1, D], BF16, tag="et1")
        nc.gpsimd.dma_start(out=et0, in_=d["enc"][bc, 0:E0, :])
        nc.gpsimd.dma_start(out=et1, in_=d["enc"][bc, E0:E, :])

        enct = []
        for i in range(ND):
            ps = pstp.tile([P, E], BF16, tag="pst")
            sl = slice(128 * i, 128 * (i + 1))
            nc.tensor.transpose(ps[:, 0:E0], et0[:, sl], ident)
            nc.tensor.transpose(ps[:, E0:E], et1[:, sl], ident[0:E1, 0:E1])
            t = enct_p.tile([P, E + 1], BF16, tag=f"e{i}")
            nc.scalar.activation(t[:, 0:E], ps[:, 0:E], mybir.ActivationFunctionType.Copy)
            esum = small.tile([P, 1], F32, tag="esum")
            nc.vector.tensor_reduce(esum, t[:, 0:E], axis=mybir.AxisListType.X, op=mybir.AluOpType.add)
            nc.scalar.mul(t[:, E : E + 1], esum, 1.0 / E)
            enct.append(t)

        # kT projection: [dout-chunk, E+1], bf16 matmul (psum accumulates f32)
        for j in range(ND):
            ps = psml.tile([P, E + 1], F32, tag="ps")
            for i in range(ND):
                nc.tensor.matmul(
                    ps,
                    lhsT=wkv[i][:, 128 * j : 128 * (j + 1)],
                    rhs=enct[i][:, 0 : E + 1],
                    start=(i == 0),
                    stop=(i == ND - 1),
                )
            kst = stage.tile([P, E + 1], BF16, tag="kst")
            nc.scalar.activation(kst, ps, mybir.ActivationFunctionType.Copy)
            nc.sync.dma_start(out=kts[bc][j], in_=kst)

        # v projection: natural [t, H*(dh+1)] bf16 with a ones column per head
        # (the AV matmul then emits the softmax denominator as row 64)
        for tch, (toff, tlen) in enumerate(((0, E0), (E0, E1))):
            vst = stage.tile([tlen, H * (DH + 1)], BF16, tag=f"vst{tch}")
            vst3 = vst.rearrange("t (h w) -> t h w", w=DH + 1)
            nc.vector.memset(vst3[:, :, DH : DH + 1], 1.0)
            for half in range(2):
                ps = pbig.tile([tlen, 512], F32, tag="pbig")
                for i in range(ND):
                    nc.tensor.matmul(
                        ps,
                        lhsT=enct[i][:, toff : toff + tlen],
                        rhs=wkv[i][:, D + 512 * half : D + 512 * (half + 1)],
                        start=(i == 0),
                        stop=(i == ND - 1),
                    )
                nc.scalar.activation(
                    vst3[:, 8 * half : 8 * (half + 1), 0:DH],
                    ps.rearrange("t (h w) -> t h w", w=DH),
                    mybir.ActivationFunctionType.Copy,
                )
            nc.sync.dma_start(out=(v0s if tch == 0 else v1s)[bc], in_=vst)

    # ---- load Wq (bf16) into the Wk/Wv slots ----
    wq = []
    for i in range(ND):
        t = wmat.tile([P, D], BF16, tag=f"w{i}")
        nc.gpsimd.dma_start(out=t, in_=d["Wq"][128 * i : 128 * (i + 1), :])
        wq.append(t)

    # ---- main iterations over (b, s-chunk) ----
    kt = {}
    v0 = {}
    v1 = {}
    ksb = {}
    for b in range(B):
        # (re)load this b's kv from scratch DRAM
        for c in range(C):
            bc = c * B + b
            for j in range(ND):
                t = ktp.tile([P, E + 1], BF16, tag=f"kt{c}_{j}")
                nc.sync.dma_start(out=t, in_=kts[bc][j])
                kt[(c, j)] = t
            v0[c] = vp.tile([E0, H * (DH + 1)], BF16, tag=f"v0{c}", name=f"v0{c}")
            nc.sync.dma_start(out=v0[c], in_=v0s[bc])
            v1[c] = vp.tile([E1, H * (DH + 1)], BF16, tag=f"v1{c}", name=f"v1{c}")
            nc.sync.dma_start(out=v1[c], in_=v1s[bc])
        # block-diagonal ksum/E columns for the pooled matmul: per (c, j) a
        # [128, H] tile whose cols 2j, 2j+1 hold kt's column E (zero elsewhere)
        for c in range(C):
            for j in range(ND):
                kb = ksb_p.tile([P, H], BF16, tag=f"ksb{c}_{j}", name=f"ksb{c}_{j}")
                nc.vector.memset(kb, 0.0)
                nc.vector.tensor_copy(kb[0:64, 2 * j : 2 * j + 1], kt[(c, j)][0:64, E : E + 1])
                nc.vector.tensor_copy(kb[64:128, 2 * j + 1 : 2 * j + 2], kt[(c, j)][64:128, E : E + 1])
                ksb[(c, j)] = kb

        for sc in range(n_sc):
            r0 = sc * S_TILE

            # --- A: xT and qT per component ---
            qt = {}
            for c in range(C):
                bc = c * B + b
                xin0 = xin_p.tile([P, D], BF16, tag="xin0")
                xin1 = xin_p.tile([P, D], BF16, tag="xin1")
                nc.gpsimd.dma_start(out=xin0, in_=d["x"][bc, r0 : r0 + 128, :])
                nc.gpsimd.dma_start(out=xin1, in_=d["x"][bc, r0 + 128 : r0 + 256, :])
                xt = []
                for i in range(ND):
                    ps = pstp.tile([P, S_TILE], BF16, tag="pst")
                    sl = slice(128 * i, 128 * (i + 1))
                    nc.tensor.transpose(ps[:, 0:128], xin0[:, sl], ident)
                    nc.tensor.transpose(ps[:, 128:256], xin1[:, sl], ident)
                    t = xt_p.tile([P, S_TILE], BF16, tag=f"xt{i}")
                    nc.scalar.activation(t, ps, mybir.ActivationFunctionType.Copy)
                    xt.append(t)
                for j in range(ND):
                    ps = psml.tile([P, S_TILE], F32, tag="ps")
                    for i in range(ND):
                        nc.tensor.matmul(
                            ps,
                            lhsT=wq[i][:, 128 * j : 128 * (j + 1)],
                            rhs=xt[i],
                            start=(i == 0),
                            stop=(i == ND - 1),
                        )
                    t = qt_p.tile([P, S_TILE], BF16, tag=f"qt{c}_{j}")
                    nc.scalar.mul(t, ps, SCALE)  # fold dh^-0.5 into q
                    qt[(c, j)] = t

            # --- B: scores, exp, pooled, AV(+denom row) per (c, h) ---
            # pooled/E via M=16 matmuls; matmul base partition must be 0/32/64,
            # so components 0-2 sit in ps_pool at 32c and component 3 in pp2.
            # den_sb collects per-(c,h) softmax denominators at rows 32c+h via
            # sbuf->sbuf DMA (engines cannot write single rows off-alignment).
            ps_pool = psst.tile([P, S_TILE], F32, tag="pp")
            ps_pool2 = psst.tile([H, S_TILE], F32, tag="pp2")
            den_sb = stats.tile([P, S_TILE], F32, tag="densb")

            def pooled_slot(c):
                return (ps_pool[32 * c : 32 * c + H, :]) if c < 3 else (ps_pool2[0:H, :])

            ao = {}
            for c in range(C):
                ao[c] = ao_p.tile([P, ND * S_TILE], BF16, tag="ao", name=f"ao{c}")
                for j in range(ND):
                    nc.tensor.matmul(
                        pooled_slot(c),
                        lhsT=ksb[(c, j)],
                        rhs=qt[(c, j)],
                        start=(j == 0),
                        stop=(j == ND - 1),
                    )
                for h in range(H):
                    j, hr = h // 2, 64 * (h % 2)
                    ch = c * H + h
                    lk = kt[(c, j)]
                    rq = qt[(c, j)][hr : hr + 64, :]
                    ps_a = psml.tile([P, S_TILE], F32, tag="ps")
                    nc.tensor.matmul(ps_a, lhsT=lk[hr : hr + 64, 0:E0], rhs=rq, start=True, stop=True)
                    ps_b = psml.tile([E1, S_TILE], F32, tag="ps")
                    nc.tensor.matmul(ps_b, lhsT=lk[hr : hr + 64, E0:E], rhs=rq, start=True, stop=True)
                    wa = w_p.tile([P, S_TILE], BF16, tag="wa")
                    nc.scalar.activation(wa, ps_a, mybir.ActivationFunctionType.Exp)
                    wb = w_p.tile([E1, S_TILE], BF16, tag="wb")
                    nc.scalar.activation(wb, ps_b, mybir.ActivationFunctionType.Exp)
                    # attention-value product (unnormalized); row 64 = denom
                    ps_av = psml.tile([DH + 1, S_TILE], F32, tag="ps")
                    v_sl = slice((DH + 1) * h, (DH + 1) * (h + 1))
                    nc.tensor.matmul(ps_av, lhsT=v0[c][:, v_sl], rhs=wa, start=True, stop=False)
                    nc.tensor.matmul(ps_av, lhsT=v1[c][:, v_sl], rhs=wb, start=False, stop=True)
                    nc.vector.tensor_copy(ao[c][hr : hr + 64, S_TILE * j : S_TILE * (j + 1)], ps_av[0:DH, :])
                    den_st = denst_p.tile([1, S_TILE], F32, tag="denst")
                    nc.vector.tensor_copy(den_st, ps_av[DH : DH + 1, :])
                    nc.sync.dma_start(out=den_sb[32 * c + h : 32 * c + h + 1, :], in_=den_st)

            # --- C: coef = softmax_c(exp(pooled)) / denom ---
            # TensorTensor ops need identical partition ranges on HW, so every
            # per-component stat lives in its own [16, S] tile at offset 0.
            ep = [stats.tile([H, S_TILE], F32, tag=f"ep{c}", name=f"ep{c}") for c in range(C)]
            rd = [stats.tile([H, S_TILE], F32, tag=f"rd{c}", name=f"rd{c}") for c in range(C)]
            coefc = [stats.tile([H, S_TILE], F32, tag=f"coefc{c}", name=f"coefc{c}") for c in range(C)]
            for c in range(C):
                nc.scalar.activation(ep[c], pooled_slot(c), mybir.ActivationFunctionType.Exp)
                nc.vector.reciprocal(rd[c], den_sb[32 * c : 32 * c + H, :])
            sc_sum = stats.tile([H, S_TILE], F32, tag="sc")
            nc.vector.tensor_add(sc_sum, ep[0], ep[1])
            nc.vector.tensor_add(sc_sum, sc_sum, ep[2])
            nc.vector.tensor_add(sc_sum, sc_sum, ep[3])
            rs = stats.tile([H, S_TILE], F32, tag="rs")
            nc.vector.reciprocal(rs, sc_sum)
            for c in range(C):
                nc.vector.tensor_mul(coefc[c], ep[c], rs)
                nc.vector.tensor_mul(coefc[c], coefc[c], rd[c])

            # --- D: scale aoT in place by coef (broadcast over dh via a DRAM
            # bounce: SBUF sources cannot have stride-0 partitions, DRAM can).
            # One [128, S] broadcast per (c, head-pair): rows 0:64 get head 2j,
            # rows 64:128 get head 2j+1, matching the ao tile layout. ---
            coef_d = dram.tile([C * H, S_TILE], F32, tag="coefd", name="coef_d", bufs=2)
            for c in range(C):
                nc.sync.dma_start(out=coef_d[H * c : H * (c + 1), :], in_=coefc[c])
            for c in range(C):
                for j in range(ND):
                    cb = coefb_p.tile([P, S_TILE], F32, tag="cb")
                    src2 = coef_d[c * H + 2 * j : c * H + 2 * j + 2, :]
                    src2 = bass.AP(
                        tensor=src2.tensor,
                        offset=src2.offset,
                        ap=[list(src2.ap[0]), [0, 64]] + [list(a) for a in src2.ap[1:]],
                    )
                    nc.sync.dma_start(out=cb, in_=src2)
                    sl_ao = ao[c][:, S_TILE * j : S_TILE * (j + 1)]
                    nc.vector.tensor_mul(sl_ao, sl_ao, cb)

            # --- E: output projection + bias + residual ---
            for c in range(C):
                bc = c * B + b
                for m in range(2):
                    rows = slice(r0 + 128 * m, r0 + 128 * (m + 1))
                    for half in range(2):
                        cols = slice(512 * half, 512 * (half + 1))
                        ps = pbig.tile([P, 512], F32, tag="pbig")
                        nc.tensor.matmul(ps, lhsT=ones_row, rhs=bo_bf[:, cols], start=True, stop=False)
                        for i in range(ND):
                            nc.tensor.matmul(
                                ps,
                                lhsT=ao[c][:, S_TILE * i + 128 * m : S_TILE * i + 128 * (m + 1)],
                                rhs=wo[i][:, cols],
                                start=False,
                                stop=(i == ND - 1),
                            )
                        xr = xr_p.tile([P, 512], F32, tag="xr")
                        nc.sync.dma_start(out=xr, in_=d["x"][bc, rows, cols])
                        oh = oh_p.tile([P, 512], F32, tag="oh")
                        nc.vector.tensor_add(oh, ps, xr)
                        nc.sync.dma_start(out=d["out"][bc, rows, cols], in_=oh)




def build_program(s_loc=S_LOC, n_cores=N_CORES):
    nc = bacc.Bacc(trn_type="TRN2", target_bir_lowering=False, debug=False, num_devices=n_cores)
    d = {
        "x": nc.dram_tensor("x", [BC, s_loc, D], F32, kind="ExternalInput").ap(),
        "enc": nc.dram_tensor("enc", [BC, E, D], F32, kind="ExternalInput").ap(),
        "Wq": nc.dram_tensor("Wq", [D, D], F32, kind="ExternalInput").ap(),
        "Wk": nc.dram_tensor("Wk", [D, D], F32, kind="ExternalInput").ap(),
        "Wv": nc.dram_tensor("Wv", [D, D], F32, kind="ExternalInput").ap(),
        "Wo": nc.dram_tensor("Wo", [D, D], F32, kind="ExternalInput").ap(),
        "bo": nc.dram_tensor("bo", [1, D], F32, kind="ExternalInput").ap(),
        "out": nc.dram_tensor("out", [BC, s_loc, D], F32, kind="ExternalOutput").ap(),
    }
    with TileContext(nc, trace_sim=False) as tc, ExitStack() as ctx:
        build_body(ctx, tc, d, s_loc)
    nc.compile()
    return nc


def make_in_maps(hidden_states, encoder_hidden_states, Wq, Wk, Wv, Wo, bo, s_loc=S_LOC, n_cores=N_CORES):
    common = {
        "enc": np.ascontiguousarray(encoder_hidden_states, dtype=np.float32),
        "Wq": np.ascontiguousarray(Wq, dtype=np.float32),
        "Wk": np.ascontiguousarray(Wk, dtype=np.float32),
        "Wv": np.ascontiguousarray(Wv, dtype=np.float32),
        "Wo": np.ascontiguousarray(Wo, dtype=np.float32),
        "bo": np.ascontiguousarray(bo, dtype=np.float32).reshape(1, D),
    }
    return [
        {"x": np.ascontiguousarray(hidden_states[:, i * s_loc : (i + 1) * s_loc, :], dtype=np.float32), **common}
        for i in range(n_cores)
    ]


_NC = None


def kernel(hidden_states, encoder_hidden_states, Wq, Wk, Wv, Wo, bo):
    global _NC
    if _NC is None:
        _NC = build_program()
    in_maps = make_in_maps(hidden_states, encoder_hidden_states, Wq, Wk, Wv, Wo, bo)
    res = run_bass_kernel_spmd(_NC, in_maps, list(range(N_CORES))).results
    out = np.concatenate([res[i]["out"] for i in range(N_CORES)], axis=1)
    return np.ascontiguousarray(out, dtype=np.float32)


if __name__ == "__main__":
    rng = np.random.default_rng(0)
    ins = {
        "hidden_states": rng.standard_normal((BC, S, D), dtype=np.float32),
        "encoder_hidden_states": rng.standard_normal((BC, E, D), dtype=np.float32),
        "Wq": rng.standard_normal((D, D), dtype=np.float32) * 0.02,
        "Wk": rng.standard_normal((D, D), dtype=np.float32) * 0.02,
        "Wv": rng.standard_normal((D, D), dtype=np.float32) * 0.02,
        "Wo": rng.standard_normal((D, D), dtype=np.float32) * 0.02,
        "bo": np.zeros((D,), np.float32),
    }
    out = kernel(**ins)
    print("out", out.shape, out.dtype, float(np.abs(out).max()))
